# revision 5
# baseline (speedup 1.0000x reference)
"""Trainium2 Bass kernel for nn_CrossAttention (dense_transformer).  v3

Sharding: 8 cores = (batch b in 0..3) x (image half in 0..1).  Each core
computes its batch's half-image (64 rows + 1 halo row each side).  All
convs and the attention output are core-local; only the tiny per-head
Gram matrices and l2-norm square-sums are AllGather'd between the two
cores sharing a batch (replica groups [[0,1],[2,3],[4,5],[6,7]]).

v3 changes over v2 (from trace analysis: DVE 69% / PE 65% / ACT 56%
busy, 279us HAM throttle, stt taps ran 1x):
  - PADW 132 -> 131 (odd row stride): center + 4 corner taps now have
    EVEN element offsets -> DVE scalar_tensor_tensor runs 2x mode
    (measured 2411ns -> ~1100ns per op).  The 4 edge taps (odd offsets
    +-1, +-131) run on the PE as diagonal-weight matmuls
    (alignment-free), same PE cost as v2's corner taps.
  - attn@v + proj fused: M_p = A_p^T @ WprojT_p computed once after
    softmax (4 tiny matmuls); the output pass is a single matmul chain
    M^T @ v reading the padded v store directly.  Removes 64 matmuls,
    64 psum-evac copies and 4 transposes.
  - k/q chunk passes interleaved (Gram pairs identical spatial tiles,
    so kT only needs chunk-local storage); Gram for chunk i issues
    after chunk i+1's k-convs so the PE queue never heads-of-line
    blocks on the transpose-copy chain.
"""
import numpy as np
import ml_dtypes

import concourse.bass as bass
import concourse.bacc as bacc
from concourse import mybir
from concourse.bass_utils import run_bass_kernel_spmd
from concourse.tile import TileContext

F32 = mybir.dt.float32
BF16 = mybir.dt.bfloat16
ALU = mybir.AluOpType
AF = mybir.ActivationFunctionType

C = 384          # channels
HEADS = 8
CP = C // HEADS  # 48
W = 128          # image width
CH = 16          # chunk center rows
CR = CH + 2      # conv rows per chunk
PADW = 131       # padded row stride (odd: corners/center get even offsets)
PADN = PADW * CR + 2         # padded buffer cols (2360)
OUT0 = 1 + PADW              # output row 0 data start (132, even)
OUT1 = OUT0 + CH * PADW      # 2228

# tap index t = (dy+1)*3 + (dx+1); tap offset = dy*PADW + dx
PE_TAPS = (1, 3, 5, 7)       # edges (odd offsets): PE diag matmuls, f32 psum
DVE_TAPS = (0, 2, 4, 6, 8)   # center+corners (even offsets): stt 2x adds


def _tap_off(t):
    dy, dx = t // 3 - 1, t % 3 - 1
    return dy * PADW + dx


def _build(HALF):
    NCH = HALF // CH
    NSP = HALF * W
    NT = NSP // 128
    XSB = CR * W
    NNT = CH * W // 128

    nc = bacc.Bacc(num_devices=8)

    x_ext = nc.declare_dram_parameter("x", [C, HALF + 2, W], BF16, isOutput=False)
    y_ext = nc.declare_dram_parameter("y", [C, HALF + 2, W], BF16, isOutput=False)
    wqT_ext = nc.declare_dram_parameter("wqT", [C, C], BF16, isOutput=False)
    wkT_ext = nc.declare_dram_parameter("wkT", [C, C], BF16, isOutput=False)
    wvT_ext = nc.declare_dram_parameter("wvT", [C, C], BF16, isOutput=False)
    wpT_ext = nc.declare_dram_parameter("wpT", [4, 96, C], BF16, isOutput=False)
    dwq_ext = nc.declare_dram_parameter("dwq", [C, 9], F32, isOutput=False)
    dwk_ext = nc.declare_dram_parameter("dwk", [C, 9], F32, isOutput=False)
    dwv_ext = nc.declare_dram_parameter("dwv", [C, 9], F32, isOutput=False)
    dg_ext = nc.declare_dram_parameter("dwdiag", [3, len(PE_TAPS), 4, 128, 128],
                                       BF16, isOutput=False)
    id_ext = nc.declare_dram_parameter("ident", [128, 128], BF16, isOutput=False)
    mask_ext = nc.declare_dram_parameter("blkmask", [96, 4 * 96], BF16,
                                         isOutput=False)
    tT_ext = nc.declare_dram_parameter("tempT", [1, C], F32, isOutput=False)
    out_ext = nc.declare_dram_parameter("out", [C, NSP], F32, isOutput=True)

    CCN = 96 * 4 * 96 + 2 * C
    cc_in = nc.dram_tensor("cc_in", [1, CCN], F32)
    cc_out = nc.dram_tensor("cc_out", [2, CCN], F32)
    rn_scr = nc.dram_tensor("rn_scr", [2, C], F32)

    with TileContext(nc) as tc:
        with tc.tile_pool(name="const", bufs=1) as cpool:
            wq_sb = [cpool.tile([128, C], BF16, tag=f"wq{k}", name=f"wq{k}") for k in range(3)]
            wk_sb = [cpool.tile([128, C], BF16, tag=f"wk{k}", name=f"wk{k}") for k in range(3)]
            wv_sb = [cpool.tile([128, C], BF16, tag=f"wv{k}", name=f"wv{k}") for k in range(3)]
            wp_sb = [cpool.tile([96, C], BF16, tag=f"wp{k}", name=f"wp{k}") for k in range(4)]
            for k in range(3):
                nc.scalar.dma_start(out=wq_sb[k][:], in_=wqT_ext[128 * k:128 * k + 128, :])
                nc.scalar.dma_start(out=wk_sb[k][:], in_=wkT_ext[128 * k:128 * k + 128, :])
                nc.scalar.dma_start(out=wv_sb[k][:], in_=wvT_ext[128 * k:128 * k + 128, :])
            for k in range(4):
                nc.scalar.dma_start(out=wp_sb[k][:], in_=wpT_ext[k])
            dwq_sb = [cpool.tile([128, 9], F32, tag=f"dwq{p}", name=f"dwq{p}") for p in range(3)]
            dwk_sb = [cpool.tile([128, 9], F32, tag=f"dwk{p}", name=f"dwk{p}") for p in range(3)]
            dwv_sb = [cpool.tile([96, 9], F32, tag=f"dwv{p}", name=f"dwv{p}") for p in range(4)]
            for p in range(3):
                nc.scalar.dma_start(out=dwq_sb[p][:], in_=dwq_ext[128 * p:128 * p + 128, :])
                nc.scalar.dma_start(out=dwk_sb[p][:], in_=dwk_ext[128 * p:128 * p + 128, :])
            for p in range(4):
                nc.scalar.dma_start(out=dwv_sb[p][:], in_=dwv_ext[96 * p:96 * p + 96, :])
            dg_sb = [[[cpool.tile([128, 128], BF16, tag=f"dg{t}{i}{p}", name=f"dg{t}{i}{p}")
                       for p in range(4 if t == 2 else 3)]
                      for i in range(len(PE_TAPS))] for t in range(3)]
            for t in range(3):
                for i in range(len(PE_TAPS)):
                    for p in range(4 if t == 2 else 3):
                        nc.scalar.dma_start(out=dg_sb[t][i][p][:], in_=dg_ext[t, i, p])
            id_sb = cpool.tile([128, 128], BF16, tag="ident", name="ident")
            nc.scalar.dma_start(out=id_sb[:], in_=id_ext[:])
            mask_sb = cpool.tile([96, 4 * 96], BF16, tag="mask", name="mask")
            nc.scalar.dma_start(out=mask_sb[:], in_=mask_ext[:])
            tT_sb = cpool.tile([1, C], F32, tag="tempT", name="tempT")
            nc.scalar.dma_start(out=tT_sb[:], in_=tT_ext[:])

            nsq_q = cpool.tile([128, 3, NCH], F32, tag="nsqq", name="nsqq")
            nsq_k = cpool.tile([128, 3, NCH], F32, tag="nsqk", name="nsqk")
            sp_sb = cpool.tile([96, 4 * 96], F32, tag="spsb", name="spsb")

            # ------------- chunk worker: conv1x1 + depthwise -------------
            def conv_dw(pool, psp, src_sb, wT_sb, dw_sb, dgt, pt, M, acc):
                """One (chunk, out-ptile).  acc: [M, PADN] bf16 pad-layout
                output tile (inter-row gap cols end up holding junk)."""
                apad = pool.tile([M, PADN], BF16, tag="apad", name="apad")
                ap3 = apad[:, 1:1 + PADW * CR].rearrange("p (r c) -> p r c", c=PADW)
                nc.vector.memset(apad[:, 0:1], 0.0)
                nc.vector.memset(ap3[:, :, 128:131], 0.0)
                nc.vector.memset(apad[:, PADN - 1:], 0.0)
                # conv1x1: bf16 matmuls, N=512 windows (f32 psum bank cap)
                for w0 in range(0, XSB, 512):
                    w1 = min(XSB, w0 + 512)
                    cps = psp.tile([M, 512], F32, tag="convps", name="convps")
                    for kt in range(3):
                        nc.tensor.matmul(
                            cps[:, 0:w1 - w0],
                            wT_sb[kt][:, M * pt:M * pt + M],
                            src_sb[kt][:, w0:w1],
                            start=(kt == 0), stop=(kt == 2),
                        )
                    r0, r1 = w0 // W, w1 // W
                    nc.any.tensor_copy(
                        ap3[:, r0:r1, 0:128],
                        cps[:, 0:w1 - w0].rearrange("p (r c) -> p r c", c=W),
                    )
                # PE edge taps (odd offsets) into f32 psum; ACT copies the
                # psum partial into acc (bf16) as the accumulator init.
                o0 = OUT0
                while o0 < OUT1:
                    wlen = min(512, OUT1 - o0)
                    dps = psp.tile([M, 512], F32, tag="dwps", name="dwps")
                    for i, t in enumerate(PE_TAPS):
                        nc.tensor.matmul(
                            dps[:, 0:wlen],
                            dg_sb[dgt][i][pt][0:M, 0:M],
                            apad[:, o0 + _tap_off(t):o0 + _tap_off(t) + wlen],
                            start=(i == 0), stop=(i == len(PE_TAPS) - 1),
                        )
                    nc.scalar.copy(acc[:, o0:o0 + wlen], dps[:, 0:wlen])
                    o0 += wlen
                # DVE center+corner taps: even offsets -> stt 2x mode
                for t in DVE_TAPS:
                    nc.vector.scalar_tensor_tensor(
                        out=acc[:, OUT0:OUT1],
                        in0=apad[:, OUT0 + _tap_off(t):OUT1 + _tap_off(t)],
                        scalar=dw_sb[pt][:, t:t + 1],
                        in1=acc[:, OUT0:OUT1],
                        op0=ALU.mult, op1=ALU.add,
                    )
                return apad

            def load_chunk(pool, ext, ci, pfx):
                tiles = []
                for kt_ in range(3):
                    t_ = pool.tile([128, XSB], BF16, tag=f"{pfx}{kt_}", name=f"{pfx}{kt_}")
                    nc.sync.dma_start(
                        out=t_[:],
                        in_=ext[128 * kt_:128 * kt_ + 128,
                                CH * ci:CH * ci + CR, :],
                    )
                    tiles.append(t_)
                return tiles

            # ============ pass 1: k and q interleaved per chunk ============
            with tc.tile_pool(name="ktp", bufs=2) as ktp, \
                 tc.tile_pool(name="sp1", bufs=2) as pool, \
                 tc.tile_pool(name="ps1", bufs=2, space="PSUM") as psp, \
                 tc.tile_pool(name="gramp", bufs=1, space="PSUM") as gramp:

                s_ps = gramp.tile([96, 4, 96], F32, tag="sps", name="sps")

                def do_tensor(src, w_sb, dw_sb, dgt, nsq, ci, tr_dst):
                    for pt in range(3):
                        acc = pool.tile([128, PADN], BF16, tag="dwacc", name="dwacc")
                        ac3 = acc[:, 1:1 + PADW * CR].rearrange(
                            "p (r c) -> p r c", c=PADW)
                        apad = conv_dw(pool, psp, src, w_sb, dw_sb, dgt,
                                       pt, 128, acc[:])
                        ap3 = apad[:, 1:1 + PADW * CR].rearrange(
                            "p (r c) -> p r c", c=PADW)
                        nc.scalar.activation(
                            ap3[:, 1:CH + 1, 0:128],
                            ac3[:, 1:CH + 1, 0:128], AF.Square,
                            accum_out=nsq[:, pt, ci:ci + 1],
                        )
                        for g in range(NNT // 4):
                            tps = psp.tile([128, 4, 128], BF16, tag="trps", name="trps")
                            for j in range(4):
                                r = 4 * g + j
                                nc.tensor.transpose(
                                    tps[:, j, :],
                                    ac3[:, 1 + r, 0:128],
                                    id_sb[:],
                                )
                            nc.any.tensor_copy(tr_dst(g, pt), tps[:])

                def gram(ci, qT, kT):
                    for j in range(NNT):
                        for p in range(4):
                            nc.tensor.matmul(
                                s_ps[:, p, :],
                                qT[:, j, 96 * p:96 * p + 96],
                                kT[:, j, 96 * p:96 * p + 96],
                                start=(ci == 0 and j == 0),
                                stop=(ci == NCH - 1 and j == NNT - 1),
                                skip_group_check=True,
                            )

                tiles_prev = None
                for ci in range(NCH):
                    ysrc = load_chunk(pool, y_ext, ci, "ysrc")
                    xsrc = load_chunk(pool, x_ext, ci, "xsrc")
                    kT = ktp.tile([128, NNT, C], BF16, tag="kTc", name="kTc")
                    qT = ktp.tile([128, NNT, C], BF16, tag="qTc", name="qTc")
                    do_tensor(ysrc, wk_sb, dwk_sb, 1, nsq_k, ci,
                              lambda g, pt, _kT=kT: _kT[
                                  :, 4 * g:4 * g + 4, 128 * pt:128 * pt + 128])
                    if tiles_prev is not None:
                        gram(ci - 1, *tiles_prev)
                    do_tensor(xsrc, wq_sb, dwq_sb, 0, nsq_q, ci,
                              lambda g, pt, _qT=qT: _qT[
                                  :, 4 * g:4 * g + 4, 128 * pt:128 * pt + 128])
                    tiles_prev = (qT, kT)
                gram(NCH - 1, *tiles_prev)
                nc.scalar.copy(
                    sp_sb[:].rearrange("p (g n) -> p g n", n=96), s_ps[:])

            # ========== v-subpass + collective + softmax + fused ==========
            # ========== (Wproj @ blockdiag(A)) @ v output pass    ==========
            nsqr_q = cpool.tile([128, 3], F32, tag="nsqrq", name="nsqrq")
            nsqr_k = cpool.tile([128, 3], F32, tag="nsqrk", name="nsqrk")
            nc.vector.tensor_reduce(out=nsqr_q[:], in_=nsq_q[:],
                                    axis=mybir.AxisListType.X, op=ALU.add)
            nc.vector.tensor_reduce(out=nsqr_k[:], in_=nsq_k[:],
                                    axis=mybir.AxisListType.X, op=ALU.add)

            ccs = [cpool.tile([96, 4 * 96], F32, tag=f"ccs{r}", name=f"ccs{r}") for r in range(2)]
            ccnq = [cpool.tile([128, 3], F32, tag=f"ccnq{r}", name=f"ccnq{r}") for r in range(2)]
            ccnk = [cpool.tile([128, 3], F32, tag=f"ccnk{r}", name=f"ccnk{r}") for r in range(2)]

            with tc.tile_pool(name="vstore", bufs=1) as vstp:
                vst = [vstp.tile([96, NCH, PADN], BF16, tag=f"vst{p}", name=f"vst{p}")
                       for p in range(4)]

                with tc.tile_pool(name="sp2", bufs=2) as pool, \
                     tc.tile_pool(name="ps2", bufs=2, space="PSUM") as psp, \
                     tc.tile_pool(name="smp", bufs=1) as smp, \
                     tc.tile_pool(name="p2", bufs=3) as p2:

                    # ---- v-subpass convs (issued before the collective
                    # so they fill the PE/DVE while gpsimd runs it) ----
                    for ci in range(NCH):
                        ysrc = load_chunk(pool, y_ext, ci, "ysrc")
                        for pt in range(4):
                            conv_dw(pool, psp, ysrc, wv_sb, dwv_sb, 2,
                                    pt, 96, vst[pt][:, ci, :])

                    # ---- collective (gpsimd-only critical section) ----
                    SLEN = 96 * 4 * 96
                    with tc.tile_critical():
                        ccsem = nc.alloc_semaphore("ccsem")
                        sv = 0
                        nc.gpsimd.dma_start(
                            out=cc_in[0, 0:SLEN].rearrange("(p n) -> p n", p=96),
                            in_=sp_sb[:]).then_inc(ccsem, 16)
                        sv += 16
                        nc.gpsimd.dma_start(
                            out=cc_in[0, SLEN:SLEN + C].rearrange(
                                "(n p) -> p n", p=128),
                            in_=nsqr_q[:]).then_inc(ccsem, 16)
                        sv += 16
                        nc.gpsimd.dma_start(
                            out=cc_in[0, SLEN + C:].rearrange(
                                "(n p) -> p n", p=128),
                            in_=nsqr_k[:]).then_inc(ccsem, 16)
                        sv += 16
                        nc.gpsimd.wait_ge(ccsem, sv)
                        nc.gpsimd.collective_compute(
                            "AllGather", ALU.bypass,
                            replica_groups=[[0, 1], [2, 3], [4, 5], [6, 7]],
                            ins=[cc_in[:].opt()],
                            outs=[cc_out[:].opt()],
                        ).then_inc(ccsem, 1)
                        sv += 1
                        nc.gpsimd.wait_ge(ccsem, sv)
                        for r in range(2):
                            nc.gpsimd.dma_start(
                                out=ccs[r][:],
                                in_=cc_out[r, 0:SLEN].rearrange(
                                    "(p n) -> p n", p=96)).then_inc(ccsem, 16)
                            sv += 16
                            nc.gpsimd.dma_start(
                                out=ccnq[r][:],
                                in_=cc_out[r, SLEN:SLEN + C].rearrange(
                                    "(n p) -> p n", p=128)).then_inc(ccsem, 16)
                            sv += 16
                            nc.gpsimd.dma_start(
                                out=ccnk[r][:],
                                in_=cc_out[r, SLEN + C:].rearrange(
                                    "(n p) -> p n", p=128)).then_inc(ccsem, 16)
                            sv += 16
                        nc.gpsimd.wait_ge(ccsem, sv)

                    # -------------------- softmax --------------------
                    s_full = smp.tile([96, 4, 96], F32, tag="sfull", name="sfull")
                    nc.vector.tensor_tensor(
                        out=s_full[:],
                        in0=ccs[0][:].rearrange("p (g n) -> p g n", n=96),
                        in1=ccs[1][:].rearrange("p (g n) -> p g n", n=96),
                        op=ALU.add)
                    rnq = smp.tile([128, 3], F32, tag="rnq", name="rnq")
                    rnk = smp.tile([128, 3], F32, tag="rnk", name="rnk")
                    nc.vector.tensor_tensor(out=rnq[:], in0=ccnq[0][:],
                                            in1=ccnq[1][:], op=ALU.add)
                    nc.vector.tensor_tensor(out=rnk[:], in0=ccnk[0][:],
                                            in1=ccnk[1][:], op=ALU.add)
                    nc.scalar.activation(rnq[:], rnq[:], AF.Sqrt)
                    nc.scalar.activation(rnk[:], rnk[:], AF.Sqrt)
                    nc.vector.tensor_scalar_max(rnq[:], rnq[:], 1e-12)
                    nc.vector.tensor_scalar_max(rnk[:], rnk[:], 1e-12)
                    nc.vector.reciprocal(rnq[:], rnq[:])
                    nc.vector.reciprocal(rnk[:], rnk[:])

                    rnqT = smp.tile([1, C], F32, tag="rnqT", name="rnqT")
                    rnkT = smp.tile([1, C], F32, tag="rnkT", name="rnkT")
                    with tc.tile_critical():
                        rsem = nc.alloc_semaphore("rsem")
                        nc.gpsimd.dma_start(
                            out=rn_scr[0, :].rearrange("(n p) -> p n", p=128),
                            in_=rnq[:]).then_inc(rsem, 16)
                        nc.gpsimd.dma_start(
                            out=rn_scr[1, :].rearrange("(n p) -> p n", p=128),
                            in_=rnk[:]).then_inc(rsem, 16)
                        nc.gpsimd.wait_ge(rsem, 32)
                        nc.gpsimd.dma_start(
                            out=rnqT[:], in_=rn_scr[0:1, :]).then_inc(rsem, 16)
                        nc.gpsimd.dma_start(
                            out=rnkT[:], in_=rn_scr[1:2, :]).then_inc(rsem, 16)
                        nc.gpsimd.wait_ge(rsem, 64)
                    nc.vector.tensor_tensor(out=rnqT[:], in0=rnqT[:],
                                            in1=tT_sb[:], op=ALU.mult)

                    outer_ps = psp.tile([96, 4, 96], F32, tag="outerps",
                                        name="outerps", bufs=1)
                    for p in range(4):
                        nc.tensor.matmul(
                            outer_ps[:, p, :],
                            rnqT[0:1, 96 * p:96 * p + 96],
                            rnkT[0:1, 96 * p:96 * p + 96],
                            start=True, stop=True,
                        )
                    logits = smp.tile([96, 4, 96], F32, tag="logits", name="logits")
                    nc.vector.tensor_tensor(out=logits[:], in0=s_full[:],
                                            in1=outer_ps[:], op=ALU.mult)
                    expv = smp.tile([96, 4 * 96], F32, tag="expv", name="expv")
                    nc.scalar.activation(
                        expv[:], logits[:].rearrange("p g n -> p (g n)"),
                        AF.Exp)
                    expm = smp.tile([96, 4, 96], F32, tag="expm", name="expm")
                    nc.vector.tensor_tensor(
                        out=expm[:],
                        in0=expv[:].rearrange("p (g n) -> p g n", n=96),
                        in1=mask_sb[:].rearrange("p (g n) -> p g n", n=96),
                        op=ALU.mult)
                    rs = smp.tile([96, 4], F32, tag="rs", name="rs")
                    nc.vector.tensor_reduce(out=rs[:], in_=expm[:],
                                            axis=mybir.AxisListType.X,
                                            op=ALU.add)
                    nc.vector.reciprocal(rs[:], rs[:])
                    attn = smp.tile([96, 4, 96], BF16, tag="attn", name="attn")
                    for p in range(4):
                        nc.vector.tensor_scalar(
                            attn[:, p, :], expm[:, p, :], rs[:, p:p + 1],
                            None, ALU.mult)

                    # fused output weights: mT[p] = A_p^T @ WprojT_p
                    mT = [smp.tile([96, C], BF16, tag=f"mT{p}", name=f"mT{p}")
                          for p in range(4)]
                    for p in range(4):
                        mtps = psp.tile([96, C], F32, tag="mtps", name="mtps",
                                        bufs=1)
                        nc.tensor.matmul(mtps[:], attn[:, p, :], wp_sb[p][:],
                                         start=True, stop=True)
                        nc.any.tensor_copy(mT[p][:], mtps[:])

                    # ---- fused (Wproj @ blockdiag(A)) @ v, per chunk ----
                    for ci in range(NCH):
                        vw = [vst[p][:, ci, 1:1 + PADW * CR].rearrange(
                            "p (r c) -> p r c", c=PADW) for p in range(4)]
                        for w2 in range(CH // 8):      # 8-row output windows
                            for half in range(2):
                                r0 = 1 + 8 * w2 + 4 * half
                                for o in range(3):
                                    pps = psp.tile([128, 512], F32,
                                                   tag="p2ps", name="pps")
                                    for kp in range(4):
                                        nc.tensor.matmul(
                                            pps[:].rearrange(
                                                "p (r c) -> p r c", c=W),
                                            mT[kp][:, 128 * o:128 * o + 128],
                                            vw[kp][:, r0:r0 + 4, 0:128],
                                            start=(kp == 0), stop=(kp == 3))
                                    osb = p2.tile([128, 512], F32, tag="osb", name="osb")
                                    nc.any.tensor_copy(osb[:], pps[:])
                                    w2g = ci * (CH // 8) + w2
                                    hw0 = 512 * half
                                    nc.sync.dma_start(
                                        out=out_ext[
                                            128 * o:128 * o + 128,
                                            1024 * w2g + hw0:1024 * w2g + hw0 + 512],
                                        in_=osb[:])
    return nc


_BUILD_CACHE = {}


def _get_program(HALF):
    if HALF not in _BUILD_CACHE:
        nc = _build(HALF)
        if not nc.is_finalized():
            nc.finalize()
        _BUILD_CACHE[HALF] = nc
    return _BUILD_CACHE[HALF]


def kernel(x, y, Wq, Wkv, Wdw, Wproj, temperature):
    B, C_, H, W_ = x.shape
    assert C_ == C and W_ == W
    HALF = H // 2
    nc = _get_program(HALF)

    f32 = np.float32
    bf16 = ml_dtypes.bfloat16
    x = np.asarray(x, f32)
    y = np.asarray(y, f32)
    Wq = np.asarray(Wq, f32)
    Wkv = np.asarray(Wkv, f32)
    Wdw = np.asarray(Wdw, f32)
    Wproj = np.asarray(Wproj, f32)
    temperature = np.asarray(temperature, f32)

    wqT = np.ascontiguousarray(Wq.T).astype(bf16)
    wkT = np.ascontiguousarray(Wkv[:C].T).astype(bf16)
    wvT = np.ascontiguousarray(Wkv[C:].T).astype(bf16)
    wpT = np.ascontiguousarray(Wproj.T.reshape(4, 96, C)).astype(bf16)
    dwq = np.ascontiguousarray(Wdw[0:C, 0].reshape(C, 9))
    dwk = np.ascontiguousarray(Wdw[C:2 * C, 0].reshape(C, 9))
    dwv = np.ascontiguousarray(Wdw[2 * C:, 0].reshape(C, 9))

    npe = len(PE_TAPS)
    dwdiag = np.zeros((3, npe, 4, 128, 128), f32)
    for i, t in enumerate(PE_TAPS):
        for p in range(3):
            dwdiag[0, i, p][np.arange(128), np.arange(128)] = \
                dwq[128 * p:128 * p + 128, t]
            dwdiag[1, i, p][np.arange(128), np.arange(128)] = \
                dwk[128 * p:128 * p + 128, t]
        for p in range(4):
            dwdiag[2, i, p][np.arange(96), np.arange(96)] = \
                dwv[96 * p:96 * p + 96, t]
    dwdiag = dwdiag.astype(bf16)

    ident = np.eye(128, dtype=bf16)
    blk = np.zeros((96, 4 * 96), f32)
    for p in range(4):
        blk[0:48, 96 * p:96 * p + 48] = 1.0
        blk[48:96, 96 * p + 48:96 * p + 96] = 1.0
    blkmask = blk.astype(bf16)
    tempT = np.repeat(temperature.reshape(HEADS), CP).reshape(1, C).astype(f32)

    in_maps = []
    for c in range(8):
        b, half = c // 2, c % 2
        r0 = half * HALF

        def shard(t):
            s = np.zeros((C, HALF + 2, W_), f32)
            s[:, 1:HALF + 1] = t[b, :, r0:r0 + HALF]
            if r0 > 0:
                s[:, 0] = t[b, :, r0 - 1]
            if r0 + HALF < H:
                s[:, HALF + 1] = t[b, :, r0 + HALF]
            return s.astype(bf16)

        in_maps.append({
            "x": shard(x), "y": shard(y),
            "wqT": wqT, "wkT": wkT, "wvT": wvT, "wpT": wpT,
            "dwq": dwq, "dwk": dwk, "dwv": dwv,
            "dwdiag": dwdiag, "ident": ident, "blkmask": blkmask,
            "tempT": tempT,
        })

    import os
    trace = bool(os.environ.get("KBENCH_TRACE"))
    kw = {}
    if trace:
        kw = dict(trace=True)
    res = run_bass_kernel_spmd(nc, in_maps, list(range(8)), **kw)
    kernel._last_result = res

    out = np.zeros((B, C, H, W_), f32)
    for c in range(8):
        b, half = c // 2, c % 2
        out[b, :, half * HALF:(half + 1) * HALF] = \
            np.asarray(res.results[c]["out"], f32).reshape(C, HALF, W_)
    return out


# revision 6
# speedup vs baseline: 1.3619x; 1.3619x over previous
"""Trainium2 Bass kernel for nn_CrossAttention (dense_transformer).  v4

Sharding: 8 cores = (batch b in 0..3) x (image half in 0..1).  Each core
computes its batch's half-image (64 rows + 1 halo row each side).  All
convs and the attention output are core-local; only the tiny per-head
Gram matrices and l2-norm square-sums are AllGather'd between the two
cores sharing a batch (replica groups [[0,1],[2,3],[4,5],[6,7]]).

v4 notes (from v2/v3 trace analysis):
  - stt has NO 2x mode on trn2 (measured 1x at any alignment); the fast
    DVE tap is tensor_scalar (4x) + tensor_tensor (2x).  PADW=131 keeps
    center+corner taps at even offsets (4x/2x eligible); the 4 edge
    taps + center run on PE as diagonal-weight matmuls (5 PE taps),
    4 corners on DVE as ts+tt pairs.
  - v processed in 3x128-channel ptiles (not 4x96): -25% on all v-side
    engine work.  The fused output weights M = Wproj @ blockdiag(A)
    are computed per 96-head-group then regrouped to 3x128 tiles with
    6 tiny SBUF->SBUF DMAs.
  - attn@v + proj fused into a single matmul chain M^T @ v.
  - collective issued between pass1 and pass2; softmax issued midway
    through the v-pass so its engine-queue position lands right after
    the collective completes; const DMAs + memsets on gpsimd; chunk
    loads prefetched one ahead; transposes in 8-wide psum groups.
"""
import numpy as np
import ml_dtypes

import concourse.bass as bass
import concourse.bacc as bacc
from concourse import mybir
from concourse.bass_utils import run_bass_kernel_spmd
from concourse.tile import TileContext

F32 = mybir.dt.float32
BF16 = mybir.dt.bfloat16
ALU = mybir.AluOpType
AF = mybir.ActivationFunctionType

C = 384          # channels
HEADS = 8
CP = C // HEADS  # 48
W = 128          # image width
CH = 16          # chunk center rows
CR = CH + 2      # conv rows per chunk
PADW = 131       # padded row stride (odd: corners/center get even offsets)
PADN = PADW * CR + 2         # padded buffer cols (2360)
OUT0 = 1 + PADW              # output row 0 data start (132, even)
OUT1 = OUT0 + CH * PADW      # 2228

# tap index t = (dy+1)*3 + (dx+1); tap offset = dy*PADW + dx
PE_TAPS = (1, 3, 4, 5, 7)    # edges (odd offsets) + center: PE diag matmuls
DVE_TAPS = (0, 2, 6, 8)      # corners (even offsets): ts 4x + tt 2x


def _tap_off(t):
    dy, dx = t // 3 - 1, t % 3 - 1
    return dy * PADW + dx


def _build(HALF):
    NCH = HALF // CH
    NSP = HALF * W
    XSB = CR * W
    NNT = CH * W // 128

    nc = bacc.Bacc(num_devices=8)

    x_ext = nc.declare_dram_parameter("x", [C, HALF + 2, W], BF16, isOutput=False)
    y_ext = nc.declare_dram_parameter("y", [C, HALF + 2, W], BF16, isOutput=False)
    wqT_ext = nc.declare_dram_parameter("wqT", [C, C], BF16, isOutput=False)
    wkT_ext = nc.declare_dram_parameter("wkT", [C, C], BF16, isOutput=False)
    wvT_ext = nc.declare_dram_parameter("wvT", [C, C], BF16, isOutput=False)
    wpT_ext = nc.declare_dram_parameter("wpT", [4, 96, C], BF16, isOutput=False)
    dwq_ext = nc.declare_dram_parameter("dwq", [C, 9], F32, isOutput=False)
    dwk_ext = nc.declare_dram_parameter("dwk", [C, 9], F32, isOutput=False)
    dwv_ext = nc.declare_dram_parameter("dwv", [C, 9], F32, isOutput=False)
    dg_ext = nc.declare_dram_parameter("dwdiag", [3, len(PE_TAPS), 3, 128, 128],
                                       BF16, isOutput=False)
    id_ext = nc.declare_dram_parameter("ident", [128, 128], BF16, isOutput=False)
    mask_ext = nc.declare_dram_parameter("blkmask", [96, 4 * 96], BF16,
                                         isOutput=False)
    tT_ext = nc.declare_dram_parameter("tempT", [1, C], F32, isOutput=False)
    out_ext = nc.declare_dram_parameter("out", [C, NSP], F32, isOutput=True)

    CCN = 96 * 4 * 96 + 2 * C
    cc_in = nc.dram_tensor("cc_in", [1, CCN], F32)
    cc_out = nc.dram_tensor("cc_out", [2, CCN], F32)
    rn_scr = nc.dram_tensor("rn_scr", [2, C], F32)

    with TileContext(nc) as tc:
        with tc.tile_pool(name="const", bufs=1) as cpool:
            wq_sb = [cpool.tile([128, C], BF16, tag=f"wq{k}", name=f"wq{k}") for k in range(3)]
            wk_sb = [cpool.tile([128, C], BF16, tag=f"wk{k}", name=f"wk{k}") for k in range(3)]
            wv_sb = [cpool.tile([128, C], BF16, tag=f"wv{k}", name=f"wv{k}") for k in range(3)]
            wp_sb = [cpool.tile([96, C], BF16, tag=f"wp{k}", name=f"wp{k}") for k in range(4)]
            for k in range(3):
                nc.gpsimd.dma_start(out=wq_sb[k][:], in_=wqT_ext[128 * k:128 * k + 128, :])
                nc.gpsimd.dma_start(out=wk_sb[k][:], in_=wkT_ext[128 * k:128 * k + 128, :])
                nc.gpsimd.dma_start(out=wv_sb[k][:], in_=wvT_ext[128 * k:128 * k + 128, :])
            for k in range(4):
                nc.gpsimd.dma_start(out=wp_sb[k][:], in_=wpT_ext[k])
            dwq_sb = [cpool.tile([128, 9], F32, tag=f"dwq{p}", name=f"dwq{p}") for p in range(3)]
            dwk_sb = [cpool.tile([128, 9], F32, tag=f"dwk{p}", name=f"dwk{p}") for p in range(3)]
            dwv_sb = [cpool.tile([128, 9], F32, tag=f"dwv{p}", name=f"dwv{p}") for p in range(3)]
            for p in range(3):
                nc.gpsimd.dma_start(out=dwq_sb[p][:], in_=dwq_ext[128 * p:128 * p + 128, :])
                nc.gpsimd.dma_start(out=dwk_sb[p][:], in_=dwk_ext[128 * p:128 * p + 128, :])
                nc.gpsimd.dma_start(out=dwv_sb[p][:], in_=dwv_ext[128 * p:128 * p + 128, :])
            dg_sb = [[[cpool.tile([128, 128], BF16, tag=f"dg{t}{i}{p}", name=f"dg{t}{i}{p}")
                       for p in range(3)]
                      for i in range(len(PE_TAPS))] for t in range(3)]
            for t in range(3):
                for i in range(len(PE_TAPS)):
                    for p in range(3):
                        nc.gpsimd.dma_start(out=dg_sb[t][i][p][:], in_=dg_ext[t, i, p])
            id_sb = cpool.tile([128, 128], BF16, tag="ident", name="ident")
            nc.gpsimd.dma_start(out=id_sb[:], in_=id_ext[:])
            mask_sb = cpool.tile([96, 4 * 96], BF16, tag="mask", name="mask")
            nc.gpsimd.dma_start(out=mask_sb[:], in_=mask_ext[:])
            tT_sb = cpool.tile([1, C], F32, tag="tempT", name="tempT")
            nc.gpsimd.dma_start(out=tT_sb[:], in_=tT_ext[:])

            nsq_q = cpool.tile([128, 3, NCH], F32, tag="nsqq", name="nsqq")
            nsq_k = cpool.tile([128, 3, NCH], F32, tag="nsqk", name="nsqk")
            sp_sb = cpool.tile([96, 4 * 96], F32, tag="spsb", name="spsb")

            # ------------- chunk worker: conv1x1 + depthwise -------------
            def conv_dw(pool, psp, src_sb, wT_sb, dw_sb, dgt, pt, acc):
                """One (chunk, out-ptile).  acc: [128, PADN] bf16 pad-layout
                output tile (inter-row gap cols end up holding junk)."""
                apad = pool.tile([128, PADN], BF16, tag="apad", name="apad")
                ap3 = apad[:, 1:1 + PADW * CR].rearrange("p (r c) -> p r c", c=PADW)
                nc.gpsimd.memset(apad[:, 0:1], 0.0)
                nc.gpsimd.memset(ap3[:, :, 128:131], 0.0)
                nc.gpsimd.memset(apad[:, PADN - 1:], 0.0)
                # conv1x1: bf16 matmuls, N=512 windows (f32 psum bank cap)
                for w0 in range(0, XSB, 512):
                    w1 = min(XSB, w0 + 512)
                    cps = psp.tile([128, 512], F32, tag="convps", name="convps")
                    for kt in range(3):
                        nc.tensor.matmul(
                            cps[:, 0:w1 - w0],
                            wT_sb[kt][:, 128 * pt:128 * pt + 128],
                            src_sb[kt][:, w0:w1],
                            start=(kt == 0), stop=(kt == 2),
                        )
                    r0, r1 = w0 // W, w1 // W
                    nc.any.tensor_copy(
                        ap3[:, r0:r1, 0:128],
                        cps[:, 0:w1 - w0].rearrange("p (r c) -> p r c", c=W),
                    )
                # PE taps (edges + center) into f32 psum; psum partial is
                # copied into acc (bf16) as the accumulator init.
                o0 = OUT0
                while o0 < OUT1:
                    wlen = min(512, OUT1 - o0)
                    dps = psp.tile([128, 512], F32, tag="dwps", name="dwps")
                    for i, t in enumerate(PE_TAPS):
                        nc.tensor.matmul(
                            dps[:, 0:wlen],
                            dg_sb[dgt][i][pt][:],
                            apad[:, o0 + _tap_off(t):o0 + _tap_off(t) + wlen],
                            start=(i == 0), stop=(i == len(PE_TAPS) - 1),
                        )
                    nc.any.tensor_copy(acc[:, o0:o0 + wlen], dps[:, 0:wlen])
                    o0 += wlen
                # DVE corner taps: ts (4x) into tmp, tt (2x) add into acc
                for t in DVE_TAPS:
                    tmp = pool.tile([128, OUT1 - OUT0], BF16, tag="dwtmp",
                                    name="dwtmp")
                    nc.vector.tensor_scalar(
                        tmp[:], apad[:, OUT0 + _tap_off(t):OUT1 + _tap_off(t)],
                        dw_sb[pt][:, t:t + 1], None, ALU.mult)
                    nc.vector.tensor_tensor(
                        out=acc[:, OUT0:OUT1], in0=tmp[:],
                        in1=acc[:, OUT0:OUT1], op=ALU.add)
                return apad

            def load_chunk(pool, ext, ci, pfx):
                tiles = []
                for kt_ in range(3):
                    t_ = pool.tile([128, XSB], BF16, tag=f"{pfx}{kt_}", name=f"{pfx}{kt_}")
                    nc.sync.dma_start(
                        out=t_[:],
                        in_=ext[128 * kt_:128 * kt_ + 128,
                                CH * ci:CH * ci + CR, :],
                    )
                    tiles.append(t_)
                return tiles

            # ============ pass 1: k and q interleaved per chunk ============
            with tc.tile_pool(name="ktp", bufs=2) as ktp, \
                 tc.tile_pool(name="sp1", bufs=2) as pool, \
                 tc.tile_pool(name="ps1", bufs=2, space="PSUM") as psp, \
                 tc.tile_pool(name="gramp", bufs=1, space="PSUM") as gramp:

                s_ps = gramp.tile([96, 4, 96], F32, tag="sps", name="sps")

                def do_tensor(src, w_sb, dw_sb, dgt, nsq, ci, tr_dst):
                    for pt in range(3):
                        acc = pool.tile([128, PADN], BF16, tag="dwacc", name="dwacc")
                        ac3 = acc[:, 1:1 + PADW * CR].rearrange(
                            "p (r c) -> p r c", c=PADW)
                        apad = conv_dw(pool, psp, src, w_sb, dw_sb, dgt,
                                       pt, acc[:])
                        ap3 = apad[:, 1:1 + PADW * CR].rearrange(
                            "p (r c) -> p r c", c=PADW)
                        nc.scalar.activation(
                            ap3[:, 1:CH + 1, 0:128],
                            ac3[:, 1:CH + 1, 0:128], AF.Square,
                            accum_out=nsq[:, pt, ci:ci + 1],
                        )
                        for g in range(NNT // 8):
                            tps = psp.tile([128, 8, 128], BF16, tag="trps", name="trps")
                            for j in range(8):
                                r = 8 * g + j
                                nc.tensor.transpose(
                                    tps[:, j, :],
                                    ac3[:, 1 + r, 0:128],
                                    id_sb[:],
                                )
                            nc.any.tensor_copy(tr_dst(g, pt), tps[:])

                def gram(ci, qT, kT):
                    for j in range(NNT):
                        for p in range(4):
                            nc.tensor.matmul(
                                s_ps[:, p, :],
                                qT[:, j, 96 * p:96 * p + 96],
                                kT[:, j, 96 * p:96 * p + 96],
                                start=(ci == 0 and j == 0),
                                stop=(ci == NCH - 1 and j == NNT - 1),
                                skip_group_check=True,
                            )

                def load_xy(ci):
                    ys = load_chunk(pool, y_ext, ci, "ysrc")
                    xs = load_chunk(pool, x_ext, ci, "xsrc")
                    return ys, xs

                cur = load_xy(0)
                tiles_prev = None
                for ci in range(NCH):
                    nxt = load_xy(ci + 1) if ci + 1 < NCH else None
                    ysrc, xsrc = cur
                    kT = ktp.tile([128, NNT, C], BF16, tag="kTc", name="kTc")
                    qT = ktp.tile([128, NNT, C], BF16, tag="qTc", name="qTc")
                    do_tensor(ysrc, wk_sb, dwk_sb, 1, nsq_k, ci,
                              lambda g, pt, _kT=kT: _kT[
                                  :, 8 * g:8 * g + 8, 128 * pt:128 * pt + 128])
                    if tiles_prev is not None:
                        gram(ci - 1, *tiles_prev)
                    do_tensor(xsrc, wq_sb, dwq_sb, 0, nsq_q, ci,
                              lambda g, pt, _qT=qT: _qT[
                                  :, 8 * g:8 * g + 8, 128 * pt:128 * pt + 128])
                    tiles_prev = (qT, kT)
                    cur = nxt
                gram(NCH - 1, *tiles_prev)
                nc.scalar.copy(
                    sp_sb[:].rearrange("p (g n) -> p g n", n=96), s_ps[:])

            # norm square-sums over chunks (tiny)
            nsqr_q = cpool.tile([128, 3], F32, tag="nsqrq", name="nsqrq")
            nsqr_k = cpool.tile([128, 3], F32, tag="nsqrk", name="nsqrk")
            nc.vector.tensor_reduce(out=nsqr_q[:], in_=nsq_q[:],
                                    axis=mybir.AxisListType.X, op=ALU.add)
            nc.vector.tensor_reduce(out=nsqr_k[:], in_=nsq_k[:],
                                    axis=mybir.AxisListType.X, op=ALU.add)

            ccs = [cpool.tile([96, 4 * 96], F32, tag=f"ccs{r}", name=f"ccs{r}") for r in range(2)]
            ccnq = [cpool.tile([128, 3], F32, tag=f"ccnq{r}", name=f"ccnq{r}") for r in range(2)]
            ccnk = [cpool.tile([128, 3], F32, tag=f"ccnk{r}", name=f"ccnk{r}") for r in range(2)]

            # ---- collective (gpsimd-only critical section), issued ----
            # ---- before pass 2 so it overlaps the v-convs          ----
            SLEN = 96 * 4 * 96
            with tc.tile_critical():
                ccsem = nc.alloc_semaphore("ccsem")
                sv = 0
                nc.gpsimd.dma_start(
                    out=cc_in[0, 0:SLEN].rearrange("(p n) -> p n", p=96),
                    in_=sp_sb[:]).then_inc(ccsem, 16)
                sv += 16
                nc.gpsimd.dma_start(
                    out=cc_in[0, SLEN:SLEN + C].rearrange(
                        "(n p) -> p n", p=128),
                    in_=nsqr_q[:]).then_inc(ccsem, 16)
                sv += 16
                nc.gpsimd.dma_start(
                    out=cc_in[0, SLEN + C:].rearrange(
                        "(n p) -> p n", p=128),
                    in_=nsqr_k[:]).then_inc(ccsem, 16)
                sv += 16
                nc.gpsimd.wait_ge(ccsem, sv)
                nc.gpsimd.collective_compute(
                    "AllGather", ALU.bypass,
                    replica_groups=[[0, 1], [2, 3], [4, 5], [6, 7]],
                    ins=[cc_in[:].opt()],
                    outs=[cc_out[:].opt()],
                ).then_inc(ccsem, 1)
                sv += 1
                nc.gpsimd.wait_ge(ccsem, sv)
                for r in range(2):
                    nc.gpsimd.dma_start(
                        out=ccs[r][:],
                        in_=cc_out[r, 0:SLEN].rearrange(
                            "(p n) -> p n", p=96)).then_inc(ccsem, 16)
                    sv += 16
                    nc.gpsimd.dma_start(
                        out=ccnq[r][:],
                        in_=cc_out[r, SLEN:SLEN + C].rearrange(
                            "(n p) -> p n", p=128)).then_inc(ccsem, 16)
                    sv += 16
                    nc.gpsimd.dma_start(
                        out=ccnk[r][:],
                        in_=cc_out[r, SLEN + C:].rearrange(
                            "(n p) -> p n", p=128)).then_inc(ccsem, 16)
                    sv += 16
                nc.gpsimd.wait_ge(ccsem, sv)

            # ============ pass 2: v convs + softmax + fused output =========
            with tc.tile_pool(name="vstore", bufs=1) as vstp:
                vst = [vstp.tile([128, NCH, PADN], BF16, tag=f"vst{p}", name=f"vst{p}")
                       for p in range(3)]

                with tc.tile_pool(name="sp2", bufs=2) as pool, \
                     tc.tile_pool(name="ps2", bufs=2, space="PSUM") as psp, \
                     tc.tile_pool(name="smp", bufs=1) as smp, \
                     tc.tile_pool(name="p2", bufs=3) as p2:

                    def v_chunk(ci, src):
                        for pt in range(3):
                            conv_dw(pool, psp, src, wv_sb, dwv_sb, 2,
                                    pt, vst[pt][:, ci, :])

                    vcur = load_chunk(pool, y_ext, 0, "vsrc")
                    for ci in range(NCH // 2):
                        vnxt = load_chunk(pool, y_ext, ci + 1, "vsrc")
                        v_chunk(ci, vcur)
                        vcur = vnxt

                    # ---- softmax (issued mid-v-pass: executes right ----
                    # ---- after the collective lands)               ----
                    s_full = smp.tile([96, 4, 96], F32, tag="sfull", name="sfull")
                    nc.vector.tensor_tensor(
                        out=s_full[:],
                        in0=ccs[0][:].rearrange("p (g n) -> p g n", n=96),
                        in1=ccs[1][:].rearrange("p (g n) -> p g n", n=96),
                        op=ALU.add)
                    rnq = smp.tile([128, 3], F32, tag="rnq", name="rnq")
                    rnk = smp.tile([128, 3], F32, tag="rnk", name="rnk")
                    nc.vector.tensor_tensor(out=rnq[:], in0=ccnq[0][:],
                                            in1=ccnq[1][:], op=ALU.add)
                    nc.vector.tensor_tensor(out=rnk[:], in0=ccnk[0][:],
                                            in1=ccnk[1][:], op=ALU.add)
                    nc.scalar.activation(rnq[:], rnq[:], AF.Sqrt)
                    nc.scalar.activation(rnk[:], rnk[:], AF.Sqrt)
                    nc.vector.tensor_scalar_max(rnq[:], rnq[:], 1e-12)
                    nc.vector.tensor_scalar_max(rnk[:], rnk[:], 1e-12)
                    nc.vector.reciprocal(rnq[:], rnq[:])
                    nc.vector.reciprocal(rnk[:], rnk[:])

                    rnqT = smp.tile([1, C], F32, tag="rnqT", name="rnqT")
                    rnkT = smp.tile([1, C], F32, tag="rnkT", name="rnkT")
                    with tc.tile_critical():
                        rsem = nc.alloc_semaphore("rsem")
                        nc.gpsimd.dma_start(
                            out=rn_scr[0, :].rearrange("(n p) -> p n", p=128),
                            in_=rnq[:]).then_inc(rsem, 16)
                        nc.gpsimd.dma_start(
                            out=rn_scr[1, :].rearrange("(n p) -> p n", p=128),
                            in_=rnk[:]).then_inc(rsem, 16)
                        nc.gpsimd.wait_ge(rsem, 32)
                        nc.gpsimd.dma_start(
                            out=rnqT[:], in_=rn_scr[0:1, :]).then_inc(rsem, 16)
                        nc.gpsimd.dma_start(
                            out=rnkT[:], in_=rn_scr[1:2, :]).then_inc(rsem, 16)
                        nc.gpsimd.wait_ge(rsem, 64)
                    nc.vector.tensor_tensor(out=rnqT[:], in0=rnqT[:],
                                            in1=tT_sb[:], op=ALU.mult)

                    outer_ps = psp.tile([96, 4, 96], F32, tag="outerps",
                                        name="outerps", bufs=1)
                    for p in range(4):
                        nc.tensor.matmul(
                            outer_ps[:, p, :],
                            rnqT[0:1, 96 * p:96 * p + 96],
                            rnkT[0:1, 96 * p:96 * p + 96],
                            start=True, stop=True,
                        )
                    logits = smp.tile([96, 4, 96], F32, tag="logits", name="logits")
                    nc.vector.tensor_tensor(out=logits[:], in0=s_full[:],
                                            in1=outer_ps[:], op=ALU.mult)
                    expv = smp.tile([96, 4 * 96], F32, tag="expv", name="expv")
                    nc.scalar.activation(
                        expv[:], logits[:].rearrange("p g n -> p (g n)"),
                        AF.Exp)
                    expm = smp.tile([96, 4, 96], F32, tag="expm", name="expm")
                    nc.vector.tensor_tensor(
                        out=expm[:],
                        in0=expv[:].rearrange("p (g n) -> p g n", n=96),
                        in1=mask_sb[:].rearrange("p (g n) -> p g n", n=96),
                        op=ALU.mult)
                    rs = smp.tile([96, 4], F32, tag="rs", name="rs")
                    nc.vector.tensor_reduce(out=rs[:], in_=expm[:],
                                            axis=mybir.AxisListType.X,
                                            op=ALU.add)
                    nc.vector.reciprocal(rs[:], rs[:])
                    attn = smp.tile([96, 4, 96], BF16, tag="attn", name="attn")
                    for p in range(4):
                        nc.vector.tensor_scalar(
                            attn[:, p, :], expm[:, p, :], rs[:, p:p + 1],
                            None, ALU.mult)

                    # fused output weights: mTg[p] = A_p^T @ WprojT_p,
                    # then regroup 4x96 -> 3x128 via SBUF->SBUF DMA
                    mTg = [smp.tile([96, C], BF16, tag=f"mTg{p}", name=f"mTg{p}")
                           for p in range(4)]
                    for p in range(4):
                        mtps = psp.tile([96, C], F32, tag="mtps", name="mtps",
                                        bufs=1)
                        nc.tensor.matmul(mtps[:], attn[:, p, :], wp_sb[p][:],
                                         start=True, stop=True)
                        nc.any.tensor_copy(mTg[p][:], mtps[:])
                    mT = [smp.tile([128, C], BF16, tag=f"mT{p}", name=f"mT{p}")
                          for p in range(3)]
                    for (g, q0, q1, t, p0) in ((0, 0, 96, 0, 0),
                                               (1, 0, 32, 0, 96),
                                               (1, 32, 96, 1, 0),
                                               (2, 0, 64, 1, 64),
                                               (2, 64, 96, 2, 0),
                                               (3, 0, 96, 2, 32)):
                        nc.sync.dma_start(out=mT[t][p0:p0 + q1 - q0, :],
                                          in_=mTg[g][q0:q1, :])

                    for ci in range(NCH // 2, NCH):
                        vnxt = (load_chunk(pool, y_ext, ci + 1, "vsrc")
                                if ci + 1 < NCH else None)
                        v_chunk(ci, vcur)
                        vcur = vnxt

                    # ---- fused (Wproj @ blockdiag(A)) @ v, per chunk ----
                    for ci in range(NCH):
                        vw = [vst[p][:, ci, 1:1 + PADW * CR].rearrange(
                            "p (r c) -> p r c", c=PADW) for p in range(3)]
                        for w2 in range(CH // 8):      # 8-row output windows
                            for half in range(2):
                                r0 = 1 + 8 * w2 + 4 * half
                                for o in range(3):
                                    pps = psp.tile([128, 512], F32,
                                                   tag="p2ps", name="pps")
                                    for kp in range(3):
                                        nc.tensor.matmul(
                                            pps[:].rearrange(
                                                "p (r c) -> p r c", c=W),
                                            mT[kp][:, 128 * o:128 * o + 128],
                                            vw[kp][:, r0:r0 + 4, 0:128],
                                            start=(kp == 0), stop=(kp == 2))
                                    osb = p2.tile([128, 512], F32, tag="osb", name="osb")
                                    nc.any.tensor_copy(osb[:], pps[:])
                                    w2g = ci * (CH // 8) + w2
                                    hw0 = 512 * half
                                    nc.sync.dma_start(
                                        out=out_ext[
                                            128 * o:128 * o + 128,
                                            1024 * w2g + hw0:1024 * w2g + hw0 + 512],
                                        in_=osb[:])
    return nc


_BUILD_CACHE = {}


def _get_program(HALF):
    if HALF not in _BUILD_CACHE:
        nc = _build(HALF)
        if not nc.is_finalized():
            nc.finalize()
        _BUILD_CACHE[HALF] = nc
    return _BUILD_CACHE[HALF]


def kernel(x, y, Wq, Wkv, Wdw, Wproj, temperature):
    B, C_, H, W_ = x.shape
    assert C_ == C and W_ == W
    HALF = H // 2
    nc = _get_program(HALF)

    f32 = np.float32
    bf16 = ml_dtypes.bfloat16
    x = np.asarray(x, f32)
    y = np.asarray(y, f32)
    Wq = np.asarray(Wq, f32)
    Wkv = np.asarray(Wkv, f32)
    Wdw = np.asarray(Wdw, f32)
    Wproj = np.asarray(Wproj, f32)
    temperature = np.asarray(temperature, f32)

    wqT = np.ascontiguousarray(Wq.T).astype(bf16)
    wkT = np.ascontiguousarray(Wkv[:C].T).astype(bf16)
    wvT = np.ascontiguousarray(Wkv[C:].T).astype(bf16)
    wpT = np.ascontiguousarray(Wproj.T.reshape(4, 96, C)).astype(bf16)
    dwq = np.ascontiguousarray(Wdw[0:C, 0].reshape(C, 9))
    dwk = np.ascontiguousarray(Wdw[C:2 * C, 0].reshape(C, 9))
    dwv = np.ascontiguousarray(Wdw[2 * C:, 0].reshape(C, 9))

    npe = len(PE_TAPS)
    dwdiag = np.zeros((3, npe, 3, 128, 128), f32)
    for i, t in enumerate(PE_TAPS):
        for p in range(3):
            dwdiag[0, i, p][np.arange(128), np.arange(128)] = \
                dwq[128 * p:128 * p + 128, t]
            dwdiag[1, i, p][np.arange(128), np.arange(128)] = \
                dwk[128 * p:128 * p + 128, t]
            dwdiag[2, i, p][np.arange(128), np.arange(128)] = \
                dwv[128 * p:128 * p + 128, t]
    dwdiag = dwdiag.astype(bf16)

    ident = np.eye(128, dtype=bf16)
    blk = np.zeros((96, 4 * 96), f32)
    for p in range(4):
        blk[0:48, 96 * p:96 * p + 48] = 1.0
        blk[48:96, 96 * p + 48:96 * p + 96] = 1.0
    blkmask = blk.astype(bf16)
    tempT = np.repeat(temperature.reshape(HEADS), CP).reshape(1, C).astype(f32)

    in_maps = []
    for c in range(8):
        b, half = c // 2, c % 2
        r0 = half * HALF

        def shard(t):
            s = np.zeros((C, HALF + 2, W_), f32)
            s[:, 1:HALF + 1] = t[b, :, r0:r0 + HALF]
            if r0 > 0:
                s[:, 0] = t[b, :, r0 - 1]
            if r0 + HALF < H:
                s[:, HALF + 1] = t[b, :, r0 + HALF]
            return s.astype(bf16)

        in_maps.append({
            "x": shard(x), "y": shard(y),
            "wqT": wqT, "wkT": wkT, "wvT": wvT, "wpT": wpT,
            "dwq": dwq, "dwk": dwk, "dwv": dwv,
            "dwdiag": dwdiag, "ident": ident, "blkmask": blkmask,
            "tempT": tempT,
        })

    import os
    trace = bool(os.environ.get("KBENCH_TRACE"))
    kw = {}
    if trace:
        kw = dict(trace=True)
    res = run_bass_kernel_spmd(nc, in_maps, list(range(8)), **kw)
    kernel._last_result = res

    out = np.zeros((B, C, H, W_), f32)
    for c in range(8):
        b, half = c // 2, c % 2
        out[b, :, half * HALF:(half + 1) * HALF] = \
            np.asarray(res.results[c]["out"], f32).reshape(C, HALF, W_)
    return out
